# revision 7
# baseline (speedup 1.0000x reference)
"""Trainium2 Bass kernel for nn_Mismatch_loss (weighted per-channel MSE loss).

Contract: kernel(**inputs) takes FULL fp32 inputs (net_out, target,
max_positiones of shape [8, 16, 384, 384]) and returns the FULL scalar
output, distributing work across 8 NeuronCores internally.

Sharding: data-parallel over batch — core b processes image b.

Math per (b, c) channel (spatial reductions over 384*384 = HW elements):
    d   = t - n
    d2  = d * d
    S1  = sum(t)        (= d1 in the reference)
    S2  = sum(d2)       (= m1 + m2)
    S3  = sum(d2 * t)   (= m1)
    loss = ALPHA*S3/(S1+eps) + (1-ALPHA)*(S2-S3)/(HW-S1+eps)
The tiny [B, C] -> scalar finalization (active-mask, count of nonzero
losses, means) runs on host from the gathered per-channel sums.

Device layout per core (v3): host uploads ONE combined tensor
x_in[128, C*2304] fp16, partition-major, where channel c occupies
columns [c*2304, (c+1)*2304) = [t(1152) | n(1152)].  Every DMA
descriptor is a 2304B contiguous run per partition.  A single HW DMA
queue ring tops out at ~270 GB/s, so the input stream is split across
two queues: qSyncDynamicHW (SP) carries every channel's t-half,
qGpSimdDynamic (Pool, software DGE) carries every channel's n-half —
measured together the rings sustain ~400+ GB/s, and both halves of a
channel arrive simultaneously, so compute sees one new channel every
~1.4us with no lumps.

Engines per channel:
  - DVE: d = t - n (per channel), p = d2 * t (adjacent channels fused
         pairwise to halve per-op overhead); fp16 2x mode
  - ACT: d2 = Square(d) with accum_out -> per-partition sum(d2) column
  - PE : per-channel column sums of t and p via one-hot fp16 weights
         (uploaded from host), accumulated across chunks/channels into
         PSUM [16, 512]; warm-up dummy matmuls ramp the PE clock out of
         its cold p-state while the first channels stream
  - PSUM partials ship to DRAM raw (host does the final 512-col sums),
    removing the on-device reduction from the critical-path tail.

Inputs are cast to fp16 on host before upload: halves HBM traffic (the
kernel is DMA-bound) at ~1e-5 relative error on the final scalar.

max_positiones is only consulted when a channel of target is exactly
all-zero (cannot happen for this problem's random-uniform inputs); that
case is handled exactly on host without shipping the tensor to devices.
"""

import os
import sys

import numpy as np

for _p in ("/opt/trn_rl_repo", "/root/.axon_site/_ro/trn_rl_repo"):
    if os.path.isdir(_p) and _p not in sys.path:
        sys.path.append(_p)

B, C, H, W = 8, 16, 384, 384
HWE = H * W          # 147456 spatial elements per channel
P = 128              # SBUF partitions
F = HWE // P         # 1152 elements per partition per channel
F2 = 2 * F           # t|n combined row per channel
CHUNKS = (512, 512, 128)   # PE matmul free-dim chunking of F
SMOOTH = 1e-6
ALPHA = 0.05

# p = d2*t instruction units: pairs early (halved DVE op overhead),
# singles at the tail (short end-of-stream dependency chain).
MUL_UNITS = [(0, 1), (2, 3), (4, 5), (6, 7), (8, 9), (10, 11), (12, 13),
             (14,), (15,)]

_CACHE = {}


def _build_bass_v3():
    import concourse.bass as bass
    import concourse.mybir as mybir

    f16 = mybir.dt.float16
    f32 = mybir.dt.float32
    Alu = mybir.AluOpType
    Act = mybir.ActivationFunctionType

    RING = 4

    nc = bass.Bass("TRN2", target_bir_lowering=False, debug=False, num_devices=1)
    x_in = nc.dram_tensor("x_in", [P, C * F2], f16, kind="ExternalInput")
    oneh_in = nc.dram_tensor("oneh", [P, C, 16], f16, kind="ExternalInput")
    # cols 0..15 = per-partition sum(d2); [0:16,16] = sum(t);
    # [0:16,17] = sum(d2*t)
    out_all = nc.dram_tensor("out_all", [P, C + 2], f32, kind="ExternalOutput")

    from contextlib import ExitStack

    with ExitStack() as ctx:
        ctx.enter_context(nc.cleanup_on_exit())
        sb = lambda name, shape, dtype: ctx.enter_context(  # noqa: E731
            nc.sbuf_tensor(name, shape, dtype)
        )
        x_all = sb("x_all", [P, C, F2], f16)
        d_sb = sb("d_sb", [P, RING, F], f16)
        d2_sb = sb("d2_sb", [P, RING, F], f16)
        p_sb = sb("p_sb", [P, RING, F], f16)
        oneh = sb("oneh_sb", [P, C, 16], f16)
        outb = sb("outb_sb", [P, C + 2], f32)
        scratch = sb("scratch_sb", [P, 1], f16)
        psum1 = ctx.enter_context(nc.psum_tensor("psum1", [16, 512], f32))
        psum3 = ctx.enter_context(nc.psum_tensor("psum3", [16, 512], f32))
        psum_d = ctx.enter_context(nc.psum_tensor("psum_d", [16, 512], f32))

        sem = nc.alloc_semaphore
        s_xo = sem("s_xo")                       # oneh upload
        s_xt = [sem(f"s_xt{c}") for c in range(C)]  # t-half per channel
        s_xn = [sem(f"s_xn{c}") for c in range(C)]  # n-half per channel
        s_d = sem("s_d")      # subs completed (per channel)
        s_sq = sem("s_sq")    # squares completed
        s_p = sem("s_p")      # muls completed (per channel)
        s_pet = sem("s_pet")  # PE t-matmul channels completed
        s_pep = sem("s_pep")  # PE p-matmul channels completed
        s_red = sem("s_red")  # final PSUM reductions completed
        s_out = sem("s_out")  # output DMA completed

        # ---- SP: oneh + all t-halves (queue 1) + output DMAs ----
        nc.sync.dma_start(oneh[:, :, :], oneh_in.ap()).then_inc(s_xo, 16)
        for c in range(C):
            nc.sync.dma_start(
                x_all[:, c, 0:F], x_in.ap()[:, c * F2 : c * F2 + F]
            ).then_inc(s_xt[c], 16)
        nc.sync.wait_ge(s_sq, C)
        nc.sync.dma_start(
            out_all.ap()[:, 0:C], outb[:, 0:C]
        ).then_inc(s_out, 16)
        nc.sync.wait_ge(s_red, 2)
        nc.sync.dma_start(
            out_all.ap()[0:16, C : C + 2], outb[0:16, C : C + 2]
        ).then_inc(s_out, 16)
        nc.sync.wait_ge(s_out, 32)

        # ---- Pool: all n-halves (software-DGE queue) ----
        for c in range(C):
            nc.gpsimd.dma_start(
                x_all[:, c, F:F2], x_in.ap()[:, c * F2 + F : (c + 1) * F2]
            ).then_inc(s_xn[c], 16)

        # ---- DVE: subs (per channel) and muls (pairs early) ----
        def emit_sub(c):
            nc.vector.wait_ge(s_xt[c], 16)
            nc.vector.wait_ge(s_xn[c], 16)
            if c >= RING:
                nc.vector.wait_ge(s_sq, c - (RING - 1))
            nc.vector.tensor_tensor(
                d_sb[:, c % RING, :],
                x_all[:, c, 0:F],
                x_all[:, c, F:F2],
                Alu.subtract,
            ).then_inc(s_d, 1)

        def emit_mul(unit):
            a, b = unit[0], unit[-1]
            n_ch = len(unit)
            nc.vector.wait_ge(s_sq, b + 1)
            if b >= RING:
                nc.vector.wait_ge(s_pep, b - (RING - 1))
            nc.vector.tensor_tensor(
                p_sb[:, a % RING : a % RING + n_ch, :],
                d2_sb[:, a % RING : a % RING + n_ch, :],
                x_all[:, a : a + n_ch, 0:F],
                Alu.mult,
            ).then_inc(s_p, n_ch)

        mul_iter = iter(MUL_UNITS)
        next_mul = next(mul_iter)
        for i in range(C):
            emit_sub(i)
            while next_mul is not None and next_mul[-1] <= i - 2:
                emit_mul(next_mul)
                next_mul = next(mul_iter, None)
        while next_mul is not None:
            emit_mul(next_mul)
            next_mul = next(mul_iter, None)

        # Final PSUM -> [16,1] reductions on DVE (ACT's Copy would need a
        # second activation-table load).
        nc.vector.wait_ge(s_pet, C)
        nc.vector.tensor_reduce(
            outb[0:16, C : C + 1], psum1[:, :],
            axis=mybir.AxisListType.X, op=Alu.add,
        ).then_inc(s_red, 1)
        nc.vector.wait_ge(s_pep, C)
        nc.vector.tensor_reduce(
            outb[0:16, C + 1 : C + 2], psum3[:, :],
            axis=mybir.AxisListType.X, op=Alu.add,
        ).then_inc(s_red, 1)

        # ---- ACT: squares with fused per-partition accumulation ----
        nc.scalar.activation(scratch[:, :], scratch[:, :], Act.Square)
        for c in range(C):
            nc.scalar.wait_ge(s_d, c + 1)
            if c >= RING:
                nc.scalar.wait_ge(s_p, c - (RING - 1))
            nc.scalar.activation(
                d2_sb[:, c % RING, :],
                d_sb[:, c % RING, :],
                Act.Square,
                accum_out=outb[:, c : c + 1],
            ).then_inc(s_sq, 1)

        # ---- PE: warm-up dummies, then one-hot column-sum matmuls ----
        # Dummy matmuls on (uninitialized) SBUF ramp the PE clock out of
        # its cold p-state while the first channels stream; their PSUM
        # bank is never read.
        for _ in range(8):
            nc.tensor.matmul(
                psum_d[:, 0:512],
                lhsT=d_sb[:, 0, 0:16],
                rhs=d2_sb[:, 0, 0:512],
                start=True,
                stop=True,
                skip_group_check=True,
            )

        def emit_t_mms(c):
            nc.tensor.wait_ge(s_xt[c], 16)
            if c == 0:
                nc.tensor.wait_ge(s_xo, 16)
            w = oneh[:, c, :]
            off = 0
            for wdt in CHUNKS:
                mm = nc.tensor.matmul(
                    psum1[:, 0:wdt],
                    lhsT=w,
                    rhs=x_all[:, c, off : off + wdt],
                    start=(c == 0 and off == 0),
                    stop=(c == C - 1 and off + wdt == F),
                    skip_group_check=True,
                )
                off += wdt
            mm.then_inc(s_pet, 1)

        def emit_p_mms(c):
            nc.tensor.wait_ge(s_p, c + 1)
            w = oneh[:, c, :]
            off = 0
            for wdt in CHUNKS:
                mm = nc.tensor.matmul(
                    psum3[:, 0:wdt],
                    lhsT=w,
                    rhs=p_sb[:, c % RING, off : off + wdt],
                    start=(c == 0 and off == 0),
                    stop=(c == C - 1 and off + wdt == F),
                    skip_group_check=True,
                )
                off += wdt
            mm.then_inc(s_pep, 1)

        PE_SKEW = 3
        for i in range(C + PE_SKEW):
            if i < C:
                emit_t_mms(i)
            if i - PE_SKEW >= 0:
                emit_p_mms(i - PE_SKEW)

        nc.all_engine_barrier()

    return nc


def _get_nc():
    key = "nc_v3"
    if key not in _CACHE:
        _CACHE[key] = _build_bass_v3()
    return _CACHE[key]


def make_oneh():
    oneh = np.zeros((P, C, 16), dtype=np.float16)
    for c in range(C):
        oneh[:, c, c] = 1.0
    return oneh


def make_in_maps(target, net_out):
    """Per-core input maps: combined [P, C*F2] fp16 partition-major tiles."""
    t16 = np.asarray(target, dtype=np.float16).reshape(B, C, P, F)
    n16 = np.asarray(net_out, dtype=np.float16).reshape(B, C, P, F)
    # x[b, p, c, 0:F] = t, x[b, p, c, F:2F] = n
    x = np.empty((B, P, C, F2), dtype=np.float16)
    x[:, :, :, 0:F] = t16.transpose(0, 2, 1, 3)
    x[:, :, :, F:F2] = n16.transpose(0, 2, 1, 3)
    x = x.reshape(B, P, C * F2)
    oneh = make_oneh()
    return [{"x_in": x[b], "oneh": oneh} for b in range(B)]


def kernel(net_out, target, max_positiones):
    from concourse import bass_utils

    nc = _get_nc()
    in_maps = make_in_maps(target, net_out)

    # The axon terminal occasionally reports the accelerator unrecoverable
    # on the first touch after a previous process ran a NEFF. The failed
    # attempt triggers recovery terminal-side, but the local PJRT client
    # stays poisoned — tear it down between retries.
    last_err = None
    for _attempt in range(4):
        try:
            res = bass_utils.run_bass_kernel_spmd(
                nc, in_maps, core_ids=list(range(8))
            )
            break
        except Exception as e:  # noqa: BLE001
            last_err = e
            import time as _time

            _time.sleep(3.0)
            try:
                import jax

                jax.clear_caches()
                jax.extend.backend.clear_backends()
            except Exception:  # noqa: BLE001
                pass
            _time.sleep(2.0)
    else:
        raise last_err

    S1 = np.empty((B, C), np.float64)
    S2 = np.empty((B, C), np.float64)
    S3 = np.empty((B, C), np.float64)
    for b in range(B):
        out = res.results[b]["out_all"].astype(np.float64)
        S1[b] = out[:16, C]
        S3[b] = out[:16, C + 1]
        S2[b] = out[:, :C].sum(axis=0)

    m1, m2, d1 = S3, S2 - S3, S1
    d2n = float(HWE) - d1
    loss = ALPHA * m1 / (d1 + SMOOTH) + (1.0 - ALPHA) * m2 / (d2n + SMOOTH)

    # active-mask: S1 != 0 implies max(target[b,c]) != 0 for non-negative
    # targets; the S1 == 0 corner is resolved exactly on host.
    active = S1 != 0.0
    for b, c in zip(*np.nonzero(~active)):
        mt = np.max(target[b, c])
        mmp = np.max(max_positiones[b, c])
        active[b, c] = not (mt == 0.0 and mmp == 0.0)

    losses = np.where(active, loss, 0.0)
    count = (losses != 0.0).sum(axis=1).astype(np.float64)
    img_losses = losses.sum(axis=1) / count
    return np.float32(img_losses.mean())


# revision 9
# speedup vs baseline: 1.0584x; 1.0584x over previous
"""Trainium2 Bass kernel for nn_Mismatch_loss (weighted per-channel MSE loss).

Contract: kernel(**inputs) takes FULL fp32 inputs (net_out, target,
max_positiones of shape [8, 16, 384, 384]) and returns the FULL scalar
output, distributing work across 8 NeuronCores internally.

Sharding: data-parallel over batch — core b processes image b.

Math per (b, c) channel (spatial reductions over 384*384 = HW elements):
    d   = t - n
    d2  = d * d
    S1  = sum(t)        (= d1 in the reference)
    S2  = sum(d2)       (= m1 + m2)
    S3  = sum(d2 * t)   (= m1)
    loss = ALPHA*S3/(S1+eps) + (1-ALPHA)*(S2-S3)/(HW-S1+eps)
The tiny [B, C] -> scalar finalization (active-mask, count of nonzero
losses, means) runs on host from the gathered per-channel sums.

Device layout per core (v3): host uploads ONE combined tensor
x_in[128, C*2304] fp16, partition-major, where channel c occupies
columns [c*2304, (c+1)*2304) = [t(1152) | n(1152)].  Every DMA
descriptor is a 2304B contiguous run per partition.  A single HW DMA
queue ring tops out at ~270 GB/s, so the input stream is split across
two queues: qSyncDynamicHW (SP) carries every channel's t-half,
qGpSimdDynamic (Pool, software DGE) carries every channel's n-half —
measured together the rings sustain ~400+ GB/s, and both halves of a
channel arrive simultaneously, so compute sees one new channel every
~1.4us with no lumps.

Engines per channel:
  - DVE: d = t - n (per channel), p = d2 * t (adjacent channels fused
         pairwise to halve per-op overhead); fp16 2x mode
  - ACT: d2 = Square(d) with accum_out -> per-partition sum(d2) column
  - PE : per-channel column sums of t and p via one-hot fp16 weights
         (uploaded from host), accumulated across chunks/channels into
         PSUM [16, 512]; warm-up dummy matmuls ramp the PE clock out of
         its cold p-state while the first channels stream
  - PSUM partials ship to DRAM raw (host does the final 512-col sums),
    removing the on-device reduction from the critical-path tail.

Inputs are cast to fp16 on host before upload: halves HBM traffic (the
kernel is DMA-bound) at ~1e-5 relative error on the final scalar.

max_positiones is only consulted when a channel of target is exactly
all-zero (cannot happen for this problem's random-uniform inputs); that
case is handled exactly on host without shipping the tensor to devices.
"""

import os
import sys

import numpy as np

for _p in ("/opt/trn_rl_repo", "/root/.axon_site/_ro/trn_rl_repo"):
    if os.path.isdir(_p) and _p not in sys.path:
        sys.path.append(_p)

B, C, H, W = 8, 16, 384, 384
HWE = H * W          # 147456 spatial elements per channel
P = 128              # SBUF partitions
F = HWE // P         # 1152 elements per partition per channel
F2 = 2 * F           # t|n combined row per channel
CHUNKS = (512, 512, 128)   # PE matmul free-dim chunking of F
SMOOTH = 1e-6
ALPHA = 0.05

# p = d2*t instruction units: pairs early (halved DVE op overhead),
# singles at the tail (short end-of-stream dependency chain).
MUL_UNITS = [(0, 1), (2, 3), (4, 5), (6, 7), (8, 9), (10, 11), (12, 13),
             (14,), (15,)]

_CACHE = {}


def _build_bass_v3():
    import concourse.bass as bass
    import concourse.mybir as mybir

    f16 = mybir.dt.float16
    f32 = mybir.dt.float32
    Alu = mybir.AluOpType
    Act = mybir.ActivationFunctionType

    RING = 4

    nc = bass.Bass("TRN2", target_bir_lowering=False, debug=False, num_devices=1)
    x_in = nc.dram_tensor("x_in", [P, C * F2], f16, kind="ExternalInput")
    oneh_in = nc.dram_tensor("oneh", [P, C, 16], f16, kind="ExternalInput")
    # cols 0..15 = per-partition sum(d2); [0:16,16] = sum(t);
    # [0:16,17] = sum(d2*t)
    out_all = nc.dram_tensor("out_all", [P, C + 2], f32, kind="ExternalOutput")

    from contextlib import ExitStack

    with ExitStack() as ctx:
        ctx.enter_context(nc.cleanup_on_exit())
        sb = lambda name, shape, dtype: ctx.enter_context(  # noqa: E731
            nc.sbuf_tensor(name, shape, dtype)
        )
        x_all = sb("x_all", [P, C, F2], f16)
        d_sb = sb("d_sb", [P, RING, F], f16)
        d2_sb = sb("d2_sb", [P, RING, F], f16)
        p_sb = sb("p_sb", [P, RING, F], f16)
        oneh = sb("oneh_sb", [P, C, 16], f16)
        outb = sb("outb_sb", [P, C + 2], f32)
        scratch = sb("scratch_sb", [P, 1], f16)
        psum1 = ctx.enter_context(nc.psum_tensor("psum1", [16, 512], f32))
        psum3 = ctx.enter_context(nc.psum_tensor("psum3", [16, 512], f32))
        psum_d = ctx.enter_context(nc.psum_tensor("psum_d", [16, 512], f32))

        sem = nc.alloc_semaphore
        s_xo = sem("s_xo")                       # oneh upload
        s_xt = [sem(f"s_xt{c}") for c in range(C)]  # input tile per channel
        s_d = sem("s_d")      # subs completed (per channel)
        s_sq = sem("s_sq")    # squares completed
        s_p = sem("s_p")      # muls completed (per channel)
        s_pet = sem("s_pet")  # PE t-matmul channels completed
        s_pep = sem("s_pep")  # PE p-matmul channels completed
        s_red = sem("s_red")  # final PSUM reductions completed
        s_out = sem("s_out")  # output DMA completed

        # ---- SP: oneh + one full-channel DMA per channel + outputs ----
        nc.sync.dma_start(oneh[:, :, :], oneh_in.ap()).then_inc(s_xo, 16)
        for c in range(C):
            nc.sync.dma_start(
                x_all[:, c, :], x_in.ap()[:, c * F2 : (c + 1) * F2]
            ).then_inc(s_xt[c], 16)
        # acc2 + Pool S1 partial columns ship once squares and Pool
        # reductions finish; the two PSUM reductions ship last.
        nc.sync.wait_ge(s_sq, C)
        nc.sync.dma_start(
            out_all.ap()[:, 0:C], outb[:, 0:C]
        ).then_inc(s_out, 16)
        nc.sync.wait_ge(s_red, 2)
        nc.sync.dma_start(
            out_all.ap()[0:16, C : C + 2],
            outb[0:16, C : C + 2],
        ).then_inc(s_out, 16)
        nc.sync.wait_ge(s_out, 32)


        # ---- DVE: subs (per channel) and muls (pairs early) ----
        def emit_sub(c):
            nc.vector.wait_ge(s_xt[c], 16)
            if c >= RING:
                nc.vector.wait_ge(s_sq, c - (RING - 1))
            nc.vector.tensor_tensor(
                d_sb[:, c % RING, :],
                x_all[:, c, 0:F],
                x_all[:, c, F:F2],
                Alu.subtract,
            ).then_inc(s_d, 1)

        def emit_mul(unit):
            a, b = unit[0], unit[-1]
            n_ch = len(unit)
            nc.vector.wait_ge(s_sq, b + 1)
            if b >= RING:
                nc.vector.wait_ge(s_pep, b - (RING - 1))
            nc.vector.tensor_tensor(
                p_sb[:, a % RING : a % RING + n_ch, :],
                d2_sb[:, a % RING : a % RING + n_ch, :],
                x_all[:, a : a + n_ch, 0:F],
                Alu.mult,
            ).then_inc(s_p, n_ch)

        mul_iter = iter(MUL_UNITS)
        next_mul = next(mul_iter)
        for i in range(C):
            emit_sub(i)
            while next_mul is not None and next_mul[-1] <= i - 2:
                emit_mul(next_mul)
                next_mul = next(mul_iter, None)
        while next_mul is not None:
            emit_mul(next_mul)
            next_mul = next(mul_iter, None)

        # Final PSUM -> [16,1] reductions on DVE (ACT's Copy would need a
        # second activation-table load).
        nc.vector.wait_ge(s_pet, C)
        nc.vector.tensor_reduce(
            outb[0:16, C : C + 1], psum1[:, :],
            axis=mybir.AxisListType.X, op=Alu.add,
        ).then_inc(s_red, 1)
        nc.vector.wait_ge(s_pep, C)
        nc.vector.tensor_reduce(
            outb[0:16, C + 1 : C + 2], psum3[:, :],
            axis=mybir.AxisListType.X, op=Alu.add,
        ).then_inc(s_red, 1)

        # ---- ACT: squares with fused per-partition accumulation ----
        nc.scalar.activation(scratch[:, :], scratch[:, :], Act.Square)
        for c in range(C):
            nc.scalar.wait_ge(s_d, c + 1)
            if c >= RING:
                nc.scalar.wait_ge(s_p, c - (RING - 1))
            nc.scalar.activation(
                d2_sb[:, c % RING, :],
                d_sb[:, c % RING, :],
                Act.Square,
                accum_out=outb[:, c : c + 1],
            ).then_inc(s_sq, 1)

        # ---- PE: one-hot column-sum matmuls ----
        def emit_t_mms(c):
            nc.tensor.wait_ge(s_xt[c], 16)
            if c == 0:
                nc.tensor.wait_ge(s_xo, 16)
            w = oneh[:, c, :]
            off = 0
            for wdt in CHUNKS:
                mm = nc.tensor.matmul(
                    psum1[:, 0:wdt],
                    lhsT=w,
                    rhs=x_all[:, c, off : off + wdt],
                    start=(c == 0 and off == 0),
                    stop=(c == C - 1 and off + wdt == F),
                    skip_group_check=True,
                )
                off += wdt
            mm.then_inc(s_pet, 1)

        def emit_p_mms(c):
            nc.tensor.wait_ge(s_p, c + 1)
            w = oneh[:, c, :]
            off = 0
            for wdt in CHUNKS:
                mm = nc.tensor.matmul(
                    psum3[:, 0:wdt],
                    lhsT=w,
                    rhs=p_sb[:, c % RING, off : off + wdt],
                    start=(c == 0 and off == 0),
                    stop=(c == C - 1 and off + wdt == F),
                    skip_group_check=True,
                )
                off += wdt
            mm.then_inc(s_pep, 1)

        PE_SKEW = 3
        for i in range(C + PE_SKEW):
            if i < C:
                emit_t_mms(i)
            if i - PE_SKEW >= 0:
                emit_p_mms(i - PE_SKEW)

        nc.all_engine_barrier()

    return nc


def _get_nc():
    key = "nc_v3"
    if key not in _CACHE:
        _CACHE[key] = _build_bass_v3()
    return _CACHE[key]


def make_oneh():
    oneh = np.zeros((P, C, 16), dtype=np.float16)
    for c in range(C):
        oneh[:, c, c] = 1.0
    return oneh


def make_in_maps(target, net_out):
    """Per-core input maps: combined [P, C*F2] fp16 partition-major tiles."""
    t16 = np.asarray(target, dtype=np.float16).reshape(B, C, P, F)
    n16 = np.asarray(net_out, dtype=np.float16).reshape(B, C, P, F)
    # x[b, p, c, 0:F] = t, x[b, p, c, F:2F] = n
    x = np.empty((B, P, C, F2), dtype=np.float16)
    x[:, :, :, 0:F] = t16.transpose(0, 2, 1, 3)
    x[:, :, :, F:F2] = n16.transpose(0, 2, 1, 3)
    x = x.reshape(B, P, C * F2)
    oneh = make_oneh()
    return [{"x_in": x[b], "oneh": oneh} for b in range(B)]


def kernel(net_out, target, max_positiones):
    from concourse import bass_utils

    nc = _get_nc()
    in_maps = make_in_maps(target, net_out)

    # The axon terminal occasionally reports the accelerator unrecoverable
    # on the first touch after a previous process ran a NEFF. The failed
    # attempt triggers recovery terminal-side, but the local PJRT client
    # stays poisoned — tear it down between retries.
    last_err = None
    for _attempt in range(4):
        try:
            res = bass_utils.run_bass_kernel_spmd(
                nc, in_maps, core_ids=list(range(8))
            )
            break
        except Exception as e:  # noqa: BLE001
            last_err = e
            import time as _time

            _time.sleep(3.0)
            try:
                import jax

                jax.clear_caches()
                jax.extend.backend.clear_backends()
            except Exception:  # noqa: BLE001
                pass
            _time.sleep(2.0)
    else:
        raise last_err

    S1 = np.empty((B, C), np.float64)
    S2 = np.empty((B, C), np.float64)
    S3 = np.empty((B, C), np.float64)
    for b in range(B):
        out = res.results[b]["out_all"].astype(np.float64)
        S1[b] = out[:16, C]
        S3[b] = out[:16, C + 1]
        S2[b] = out[:, :C].sum(axis=0)

    m1, m2, d1 = S3, S2 - S3, S1
    d2n = float(HWE) - d1
    loss = ALPHA * m1 / (d1 + SMOOTH) + (1.0 - ALPHA) * m2 / (d2n + SMOOTH)

    # active-mask: S1 != 0 implies max(target[b,c]) != 0 for non-negative
    # targets; the S1 == 0 corner is resolved exactly on host.
    active = S1 != 0.0
    for b, c in zip(*np.nonzero(~active)):
        mt = np.max(target[b, c])
        mmp = np.max(max_positiones[b, c])
        active[b, c] = not (mt == 0.0 and mmp == 0.0)

    losses = np.where(active, loss, 0.0)
    count = (losses != 0.0).sum(axis=1).astype(np.float64)
    img_losses = losses.sum(axis=1) / count
    return np.float32(img_losses.mean())
